# revision 51
# baseline (speedup 1.0000x reference)
"""Trainium2 Bass kernel: causal multi-head group attention (GQA) with RoPE.

Full-input contract: kernel(**inputs) takes the unsharded inputs and returns
the full output. Internally shards across 8 NeuronCores:
  core c -> (batch b = c // 4, head-group g = c % 4)
Each core computes 4 q heads + their single kv group end-to-end (QKV proj,
RoPE, causal attention, row-parallel out-proj partial). The host unshard sums
the 4 head-group partials per batch and adds the output bias.

v2 design (vs the fp32r baseline):
 - bf16 operands everywhere on device (PSUM accumulation stays fp32):
   halves DMA traffic, doubles DVE throughput, enables fast weight loads.
 - RoPE via signed-permutation matmul: q' = q*sin + P@(q*sin) where
   P is the signed half-swap (uses sin[d] == sin[d+64]); the sin-multiply
   doubles as the PSUM evacuation, so RoPE costs one extra DVE add + one
   small PE matmul per tile instead of 2 SBUF-swap DMAs + 3 DVE ops.
 - Out-proj partials are DMA'd straight from PSUM to DRAM (no bias add /
   no SBUF staging on device; bias is added in the host unshard).
 - Softmax denominator broadcast via gpsimd partition_broadcast instead of
   a PE ones-outer-product + DVE copy.
 - DMA dispatch is spread across the SP and Pool queues and consolidated
   into few large strided descriptors (dispatch cost is ~0.6us/DMA on the
   issuing sequencer).
 - Single fused emission loop: per 512-seq pass, QKV proj + RoPE ->
   attention chunk -> out-proj chunk, so each phase's latency bubbles are
   filled by its neighbors' matmuls and the PE stays warm.
"""

import os
import sys
from contextlib import ExitStack, nullcontext
from math import sqrt

for _p in ("/opt/trn_rl_repo", "/root/.axon_site/_ro/trn_rl_repo"):
    if os.path.isdir(_p) and _p not in sys.path:
        sys.path.insert(0, _p)

import numpy as np
import ml_dtypes
import concourse.bacc as bacc
import concourse.tile as tile
import concourse.mybir as mybir
from concourse.bass_utils import run_bass_kernel_spmd

F32 = mybir.dt.float32
F32R = mybir.dt.float32r
BF16 = mybir.dt.bfloat16
EXP = mybir.ActivationFunctionType.Exp
COPY = mybir.ActivationFunctionType.Copy
NPBF16 = ml_dtypes.bfloat16

from concourse.engine_type import EngineType
HINT_ENGINES = (EngineType.PE, EngineType.Activation, EngineType.DVE,
                EngineType.SP, EngineType.Pool)
STAGGER = os.environ.get("KSTAGGER", "0") == "1"
KCUT = os.environ.get("KCUT", "full")

N_CORES = 8
TP = 4            # head-group parallel degree (within one batch element)
BATCH = 2
D = 128           # head dim
NHL = 4           # q heads per core
ROPE_BASE = 10000.0

S_FULL = 2048     # context length
E_FULL = 2048     # model dim


def build_program(S, E, QC=512, PW=512, n_cores=N_CORES, reps=1):
    assert QC == PW
    EC = E // 128     # contraction chunks over model dim
    NKI = S // 128    # k tiles
    NSP = S // PW     # passes (also attention q-chunks: QC == PW)

    nc = bacc.Bacc("TRN2", target_bir_lowering=False, debug=False,
                   num_devices=n_cores)

    # Every input is pre-laid-out on the host to its exact SBUF layout, so
    # each load is one strided DMA with multi-KB contiguous lines.
    xs = nc.dram_tensor("xs", [(S // PW) * 128, EC * PW], BF16,
                        kind="ExternalInput").ap()
    Wq = nc.dram_tensor("Wq", [128, EC * NHL * D], BF16, kind="ExternalInput").ap()
    Wk = nc.dram_tensor("Wk", [128, EC * D], BF16, kind="ExternalInput").ap()
    Wv = nc.dram_tensor("Wv", [128, EC * D], BF16, kind="ExternalInput").ap()
    Wo = nc.dram_tensor("Wo", [128, NHL * E], BF16, kind="ExternalInput").ap()
    sinT = nc.dram_tensor("sinT", [D, S], BF16, kind="ExternalInput").ap()
    mdiag = nc.dram_tensor("mdiag", [128, 128], BF16, kind="ExternalInput").ap()
    ones_col = nc.dram_tensor("ones_col", [128, 1], BF16, kind="ExternalInput").ap()
    ones_row = nc.dram_tensor("ones_row", [1, 128], F32R, kind="ExternalInput").ap()
    ident = nc.dram_tensor("ident", [128, 128], BF16, kind="ExternalInput").ap()
    swapsgn = nc.dram_tensor("swapsgn", [128, 128], BF16, kind="ExternalInput").ap()
    out = nc.dram_tensor("out", [S, E], BF16, kind="ExternalOutput").ap()

    scale = 1.0 / sqrt(D)

    with tile.TileContext(nc) as tc, \
         (tc.For_i(0, reps, 1, hint_engines=HINT_ENGINES,
                   staggered_reset=STAGGER)
          if reps > 1 else nullcontext()), \
         ExitStack() as top:
        pers = top.enter_context(tc.tile_pool(name="pers", bufs=1))
        qT_sb = [pers.tile([128, S], BF16, tag=f"qT{h}", name=f"qT{h}")
                 for h in range(NHL)]
        kT_sb = pers.tile([128, S], BF16, name="kT_sb")
        v_sb = [pers.tile([128, D], BF16, tag=f"v{i}", name=f"v{i}")
                for i in range(NKI)]
        yT_sb = [pers.tile([128, S], BF16, tag=f"yT{h}", name=f"yT{h}")
                 for h in range(NHL)]

        psum = top.enter_context(tc.tile_pool(name="psum", bufs=1, space="PSUM"))
        hot = top.enter_context(tc.tile_pool(name="hot", bufs=1))
        wts = top.enter_context(tc.tile_pool(name="wts", bufs=1))

        mw_sb = hot.tile([128, 128], BF16, tag="mw", name="mw_sb")
        onesc = hot.tile([128, 1], BF16, tag="onesc", name="onesc")
        onesr = hot.tile([1, 128], F32R, tag="onesr", name="onesr")
        sin_sb = wts.tile([128, S], BF16, tag="sin", name="sin_sb")
        ident_sb = wts.tile([128, 128], BF16, tag="ident", name="ident_sb")
        swap_sb = wts.tile([128, 128], BF16, tag="swap", name="swap_sb")
        wq = wts.tile([128, EC * NHL * D], BF16, tag="wq", name="wq")
        wk = wts.tile([128, EC * D], BF16, tag="wk", name="wk")
        wv = wts.tile([128, EC * D], BF16, tag="wv", name="wv")
        wo = wts.tile([128, NHL * E], BF16, tag="wo", name="wo")

        xpool = top.enter_context(tc.tile_pool(name="xt", bufs=2))

        # Startup-critical DMA ordering: the first x e-groups and wq feed
        # the first proj chains, so they lead both HWDGE queues in
        # consumption order; later-needed tensors queue behind. Only cheap
        # constants ride the (slow ucode) Pool queue.
        GW = 4 * PW  # columns per x e-group piece
        xt0 = xpool.tile([128, EC * PW], BF16, tag="xt", name="xt0")
        for g in range(4):
            eng = nc.sync if g % 2 == 0 else nc.scalar
            eng.dma_start(xt0[:, GW * g:GW * (g + 1)],
                          xs[0:128, GW * g:GW * (g + 1)])
            eng.dma_start(wq[:, 2048 * g:2048 * (g + 1)],
                          Wq[:, 2048 * g:2048 * (g + 1)])
        nc.sync.dma_start(sin_sb[:], sinT[:])
        nc.scalar.dma_start(wk[:], Wk[:])
        nc.sync.dma_start(wv[:], Wv[:])
        nc.gpsimd.dma_start(swap_sb[:], swapsgn[:])
        nc.gpsimd.dma_start(ident_sb[:], ident[:])
        nc.gpsimd.dma_start(mw_sb[:], mdiag[:])
        nc.gpsimd.dma_start(onesc[:], ones_col[:])
        nc.gpsimd.dma_start(onesr[:], ones_row[:])
        nc.scalar.dma_start(wo[:, 0:NHL * E // 2], Wo[:, 0:NHL * E // 2])
        nc.sync.dma_start(wo[:, NHL * E // 2:], Wo[:, NHL * E // 2:])

        for sp in range(NSP):
            if sp > 0 and reps > 1 and STAGGER:
                tc.stage_boundary()
            lo, hi = PW * sp, PW * (sp + 1)
            W = slice(lo, hi)

            # ---- x slab load (4 consolidated strided DMAs) ----
            if sp == 0:
                xt = xt0
            else:
                xt = xpool.tile([128, EC * PW], BF16, tag="xt", name=f"xt{sp}")
                for g in range(4):
                    nc.sync.dma_start(xt[:, GW * g:GW * (g + 1)],
                                      xs[128 * sp:128 * (sp + 1),
                                         GW * g:GW * (g + 1)])

            def xsl(e):
                return xt[:, PW * e:PW * (e + 1)]

            # ---- QKV projections + RoPE ----
            # Phase A: all six proj chains back-to-back on the PE; each is
            # evacuated by a DVE sin-multiply (the RoPE first term).
            # Phase B: the five rot matmuls + DVE adds. By then every
            # sin-multiply has long finished, so the PE never waits on DVE.
            # k chain FIRST: its sin-mul evacuation is the only gate
            # between projection and every score matmul of this chunk
            ps = psum.tile([128, PW], F32, tag="proj", bufs=2, name=f"psk{sp}")
            for e in range(EC):
                nc.tensor.matmul(ps[:], wk[:, D * e:D * (e + 1)], xsl(e),
                                 start=(e == 0), stop=(e == EC - 1))
            nc.vector.tensor_mul(kT_sb[:, W], ps[:], sin_sb[:, W])
            for h in range(NHL):
                ps = psum.tile([128, PW], F32, tag="proj", bufs=2,
                               name=f"psq{sp}_{h}")
                for e in range(EC):
                    nc.tensor.matmul(
                        ps[:],
                        wq[:, NHL * D * e + D * h:NHL * D * e + D * (h + 1)],
                        xsl(e), start=(e == 0), stop=(e == EC - 1))
                nc.vector.tensor_mul(qT_sb[h][:, W], ps[:], sin_sb[:, W])
            ps = psum.tile([128, PW], F32, tag="proj", bufs=2, name=f"psv{sp}")
            for e in range(EC):
                nc.tensor.matmul(ps[:], wv[:, D * e:D * (e + 1)], xsl(e),
                                 start=(e == 0), stop=(e == EC - 1))
            vstage = hot.tile([128, PW], BF16, tag="vstage", bufs=4,
                              name=f"vst{sp}")
            nc.scalar.activation(vstage[:], ps[:], COPY)
            for j in range(4):
                vt = psum.tile([128, 128], BF16, tag="proj", bufs=2,
                               name=f"vtr{sp}_{j}")
                nc.tensor.transpose(vt[:], vstage[:, 128 * j:128 * (j + 1)],
                                    ident_sb[:])
                nc.scalar.activation(v_sb[4 * sp + j][:], vt[:], COPY)

            if KCUT == "proj":
                continue
            # ---- causal attention for q-chunk qj == sp ----
            qj = sp
            nki_hi = (qj + 1) * QC // 128
            # Heads in interleaved PAIRS: per k-tile step the two heads'
            # score matmuls / exps / PV matmuls alternate so one head's ACT
            # exp hides under the other's PE work.
            for hp in (0, 2):
                ypss, rss = [], []
                for h in (hp, hp + 1):
                    ypss.append(psum.tile([128, QC], F32, tag="yps", bufs=2,
                                          name=f"yps{h}_{qj}"))
                    rss.append(hot.tile([128, QC], BF16, tag="rs", bufs=4,
                                        name=f"rs{h}_{qj}"))
                for ki in range(nki_hi):
                    off = 128 * ki - QC * qj
                    qlo = max(0, off)
                    for hj, h in enumerate((hp, hp + 1)):
                        st = psum.tile([128, QC], F32, tag="st", bufs=2,
                                       name=f"st{h}_{qj}_{ki}")
                        nc.tensor.matmul(
                            st[:, qlo:QC], kT_sb[:, 128 * ki:128 * (ki + 1)],
                            qT_sb[h][:, QC * qj + qlo:QC * (qj + 1)],
                            start=True, stop=True)
                        if ki == 0:
                            # exp writes the denominator accumulator
                            # directly (qlo == 0 at ki == 0): no DVE copy
                            pt = rss[hj]
                        else:
                            pt = hot.tile([128, QC], BF16, tag="pt", bufs=8,
                                          name=f"pt{h}_{qj}_{ki}")
                        nc.scalar.activation(pt[:, qlo:QC], st[:, qlo:QC],
                                             EXP, scale=scale)
                        if off >= 0:
                            nc.vector.tensor_mul(pt[:, qlo:qlo + 128],
                                                 pt[:, qlo:qlo + 128],
                                                 mw_sb[:])
                        if ki != 0:
                            nc.vector.tensor_add(rss[hj][:, qlo:QC],
                                                 rss[hj][:, qlo:QC],
                                                 pt[:, qlo:QC])
                        nc.tensor.matmul(ypss[hj][:, qlo:QC], v_sb[ki][:],
                                         pt[:, qlo:QC], start=(ki == 0),
                                         stop=(ki == nki_hi - 1))
                for hj, h in enumerate((hp, hp + 1)):
                    yps, rs = ypss[hj], rss[hj]
                    rsum = psum.tile([1, QC], F32, tag="st", bufs=2,
                                     name=f"rsum{h}_{qj}")
                    nc.tensor.matmul(rsum[:], onesc[:], rs[:],
                                     start=True, stop=True)
                    rinv = hot.tile([1, QC], F32, tag="rinv", bufs=4,
                                    name=f"rinv{h}_{qj}")
                    with nc.allow_low_precision(reason="softmax denominator"):
                        nc.vector.reciprocal_approx_fast(rinv[:], rsum[:])
                    rb = hot.tile([128, QC], F32, tag="rb", bufs=4,
                                  name=f"rb{h}_{qj}")
                    nc.gpsimd.partition_broadcast(rb[:], rinv[:])
                    nc.vector.tensor_mul(yT_sb[h][:, QC * qj:QC * (qj + 1)],
                                         yps[:], rb[:])

            if KCUT == "projattn":
                continue
            # ---- out-proj partial for this chunk ----
            # PSUM is evacuated by ACT copies (ACT has headroom; DVE does
            # not), staged bf16 in SBUF, then one row-store DMA per si.
            for si in range(4 * sp, 4 * sp + 4):
                osb = hot.tile([128, E], BF16, tag="osb", bufs=3,
                               name=f"osb{si}")
                for nj in range(E // 512):
                    ops = psum.tile([128, 512], F32, tag="ops", bufs=2,
                                    name=f"ops{si}_{nj}")
                    for h in range(NHL):
                        nc.tensor.matmul(
                            ops[:], yT_sb[h][:, 128 * si:128 * (si + 1)],
                            wo[:, E * h + 512 * nj:E * h + 512 * (nj + 1)],
                            start=(h == 0), stop=(h == NHL - 1))
                    if nj % 2 == 0:
                        nc.scalar.activation(osb[:, 512 * nj:512 * (nj + 1)],
                                             ops[:], COPY)
                    else:
                        nc.vector.tensor_copy(osb[:, 512 * nj:512 * (nj + 1)],
                                              ops[:])
                nc.sync.dma_start(out[128 * si:128 * (si + 1), :], osb[:])

    nc.compile()
    return nc


def make_consts(S):
    """Host-precomputed constants (rope sin table, masks, permutation)."""
    rope_dim = D // 2
    j = np.arange(rope_dim, dtype=np.float64)
    thetas = 1.0 / ROPE_BASE ** (2.0 * j / rope_dim)
    positions = np.arange(S, dtype=np.float64)
    angles = positions[:, None] * thetas[None, :]
    sin = np.sin(np.concatenate([angles, angles], axis=1)).astype(np.float32)
    sinT = np.ascontiguousarray(sin.T)                       # [D, S]

    k_idx = np.arange(128)[:, None]
    c_idx = np.arange(128)[None, :]
    mdiag = (k_idx <= c_idx).astype(np.float32)

    # swapsgn[p, i] as matmul lhsT: rot[i] = sum_p swapsgn[p, i] * qs[p]
    #   i < 64:  rot[i] = -qs[i + 64]
    #   i >= 64: rot[i] = +qs[i - 64]
    swapsgn = np.zeros((128, 128), np.float32)
    i_lo = np.arange(64)
    swapsgn[i_lo + 64, i_lo] = -1.0
    swapsgn[i_lo, i_lo + 64] = 1.0

    return {
        "sinT": sinT.astype(NPBF16),
        "mdiag": mdiag.astype(NPBF16),
        "ones_col": np.ones((128, 1), np.float32).astype(NPBF16),
        "ones_row": np.ones((1, 128), np.float32),
        "ident": np.eye(128, dtype=np.float32).astype(NPBF16),
        "swapsgn": swapsgn.astype(NPBF16),
    }


def make_in_maps(x, Wq, Wk, Wv, Wo, S, E, QC=512, bo=None):
    """Shard full inputs into the 8 per-core input maps (bf16 on device)."""
    consts = make_consts(S)
    in_maps = []
    for c in range(N_CORES):
        b, g = c // TP, c % TP
        m = dict(consts)
        EC = E // 128
        NSP = S // QC
        # xs[128*sp + p, e*QC + mm] = x[b][QC*sp + mm, 128*e + p]
        xb = x[b].astype(NPBF16)                       # [S, E]
        xs = (xb.reshape(NSP, QC, EC, 128)
                .transpose(0, 3, 2, 1)                 # [NSP, 128, EC, QC]
                .reshape(NSP * 128, EC * QC))
        m["xs"] = np.ascontiguousarray(xs)
        # RoPE rotation folded into the weights: the reference computes
        # q' = q*sin + rot(q)*sin (same sin factor on both terms), so
        # q' = (x @ (W + W@P)) * sin with P the signed half-swap. Applied
        # per 128-wide head block in fp32 before the bf16 cast.
        def fold_rot(Wb):
            Wb = Wb.astype(np.float32)
            out = Wb.copy()
            nb = Wb.shape[1] // D
            for b_ in range(nb):
                blk = Wb[:, D * b_:D * (b_ + 1)]
                rot = np.concatenate([-blk[:, 64:], blk[:, :64]], axis=1)
                out[:, D * b_:D * (b_ + 1)] = blk + rot
            return out

        # wq[p, e*(4D) + n] = Wq'[128*e + p, 4D*g + n]
        wqs = (fold_rot(Wq[:, NHL * D * g:NHL * D * (g + 1)]).astype(NPBF16)
               .reshape(EC, 128, NHL * D).transpose(1, 0, 2)
               .reshape(128, EC * NHL * D))
        m["Wq"] = np.ascontiguousarray(wqs)
        wks = (fold_rot(Wk[:, D * g:D * (g + 1)]).astype(NPBF16)
               .reshape(EC, 128, D).transpose(1, 0, 2).reshape(128, EC * D))
        m["Wk"] = np.ascontiguousarray(wks)
        wvs = (Wv[:, D * g:D * (g + 1)].astype(NPBF16)
               .reshape(EC, 128, D).transpose(1, 0, 2).reshape(128, EC * D))
        m["Wv"] = np.ascontiguousarray(wvs)
        # wo[p, h*E + n] = Wo[4D*g + 128*h + p, n]
        wos = (Wo[NHL * D * g:NHL * D * (g + 1), :].astype(NPBF16)
               .reshape(NHL, 128, E).transpose(1, 0, 2).reshape(128, NHL * E))
        m["Wo"] = np.ascontiguousarray(wos)
        in_maps.append(m)
    return in_maps


_CACHE = {}


def _compiled_full():
    if "nc" not in _CACHE:
        _CACHE["nc"] = build_program(S_FULL, E_FULL)
    return _CACHE["nc"]


def kernel(x, Wq, Wk, Wv, Wo, bo):
    nc = _compiled_full()
    in_maps = make_in_maps(x, Wq, Wk, Wv, Wo, S_FULL, E_FULL, bo=bo)
    res = run_bass_kernel_spmd(nc, in_maps, list(range(N_CORES)))
    # unshard the row-parallel out-proj: sum the 4 head-group partials + bias
    out = np.zeros((BATCH, S_FULL, E_FULL), np.float32)
    for c in range(N_CORES):
        out[c // TP] += res.results[c]["out"].astype(np.float32)
    out += bo.astype(np.float32)[None, None, :]
    return out


# revision 52
# speedup vs baseline: 1.0333x; 1.0333x over previous
"""Trainium2 Bass kernel: causal multi-head group attention (GQA) with RoPE.

Full-input contract: kernel(**inputs) takes the unsharded inputs and returns
the full output. Internally shards across 8 NeuronCores:
  core c -> (batch b = c // 4, head-group g = c % 4)
Each core computes 4 q heads + their single kv group end-to-end (QKV proj,
RoPE, causal attention, row-parallel out-proj partial). The host unshard sums
the 4 head-group partials per batch and adds the output bias.

v2 design (vs the fp32r baseline):
 - bf16 operands everywhere on device (PSUM accumulation stays fp32):
   halves DMA traffic, doubles DVE throughput, enables fast weight loads.
 - RoPE via signed-permutation matmul: q' = q*sin + P@(q*sin) where
   P is the signed half-swap (uses sin[d] == sin[d+64]); the sin-multiply
   doubles as the PSUM evacuation, so RoPE costs one extra DVE add + one
   small PE matmul per tile instead of 2 SBUF-swap DMAs + 3 DVE ops.
 - Out-proj partials are DMA'd straight from PSUM to DRAM (no bias add /
   no SBUF staging on device; bias is added in the host unshard).
 - Softmax denominator broadcast via gpsimd partition_broadcast instead of
   a PE ones-outer-product + DVE copy.
 - DMA dispatch is spread across the SP and Pool queues and consolidated
   into few large strided descriptors (dispatch cost is ~0.6us/DMA on the
   issuing sequencer).
 - Single fused emission loop: per 512-seq pass, QKV proj + RoPE ->
   attention chunk -> out-proj chunk, so each phase's latency bubbles are
   filled by its neighbors' matmuls and the PE stays warm.
"""

import os
import sys
from contextlib import ExitStack, nullcontext
from math import sqrt

for _p in ("/opt/trn_rl_repo", "/root/.axon_site/_ro/trn_rl_repo"):
    if os.path.isdir(_p) and _p not in sys.path:
        sys.path.insert(0, _p)

import numpy as np
import ml_dtypes
import concourse.bacc as bacc
import concourse.tile as tile
import concourse.mybir as mybir
from concourse.bass_utils import run_bass_kernel_spmd

F32 = mybir.dt.float32
F32R = mybir.dt.float32r
BF16 = mybir.dt.bfloat16
EXP = mybir.ActivationFunctionType.Exp
COPY = mybir.ActivationFunctionType.Copy
NPBF16 = ml_dtypes.bfloat16

from concourse.engine_type import EngineType
HINT_ENGINES = (EngineType.PE, EngineType.Activation, EngineType.DVE,
                EngineType.SP, EngineType.Pool)
STAGGER = os.environ.get("KSTAGGER", "0") == "1"
KCUT = os.environ.get("KCUT", "full")

N_CORES = 8
TP = 4            # head-group parallel degree (within one batch element)
BATCH = 2
D = 128           # head dim
NHL = 4           # q heads per core
ROPE_BASE = 10000.0

S_FULL = 2048     # context length
E_FULL = 2048     # model dim


def build_program(S, E, QC=512, PW=512, n_cores=N_CORES, reps=1):
    assert QC == PW
    EC = E // 128     # contraction chunks over model dim
    NKI = S // 128    # k tiles
    NSP = S // PW     # passes (also attention q-chunks: QC == PW)

    nc = bacc.Bacc("TRN2", target_bir_lowering=False, debug=False,
                   num_devices=n_cores)

    # Every input is pre-laid-out on the host to its exact SBUF layout, so
    # each load is one strided DMA with multi-KB contiguous lines.
    xs = nc.dram_tensor("xs", [(S // PW) * 128, EC * PW], BF16,
                        kind="ExternalInput").ap()
    Wq = nc.dram_tensor("Wq", [128, EC * NHL * D], BF16, kind="ExternalInput").ap()
    Wk = nc.dram_tensor("Wk", [128, EC * D], BF16, kind="ExternalInput").ap()
    Wv = nc.dram_tensor("Wv", [128, EC * D], BF16, kind="ExternalInput").ap()
    Wo = nc.dram_tensor("Wo", [128, NHL * E], BF16, kind="ExternalInput").ap()
    sinT = nc.dram_tensor("sinT", [D, S], BF16, kind="ExternalInput").ap()
    mdiag = nc.dram_tensor("mdiag", [128, 128], BF16, kind="ExternalInput").ap()
    ones_col = nc.dram_tensor("ones_col", [128, 1], BF16, kind="ExternalInput").ap()
    ones_row = nc.dram_tensor("ones_row", [1, 128], F32R, kind="ExternalInput").ap()
    ident = nc.dram_tensor("ident", [128, 128], BF16, kind="ExternalInput").ap()
    swapsgn = nc.dram_tensor("swapsgn", [128, 128], BF16, kind="ExternalInput").ap()
    out = nc.dram_tensor("out", [S, E], BF16, kind="ExternalOutput").ap()

    scale = 1.0 / sqrt(D)

    with tile.TileContext(nc) as tc, \
         (tc.For_i(0, reps, 1, hint_engines=HINT_ENGINES,
                   staggered_reset=STAGGER)
          if reps > 1 else nullcontext()), \
         ExitStack() as top:
        pers = top.enter_context(tc.tile_pool(name="pers", bufs=1))
        qT_sb = [pers.tile([128, S], BF16, tag=f"qT{h}", name=f"qT{h}")
                 for h in range(NHL)]
        kT_sb = pers.tile([128, S], BF16, name="kT_sb")
        v_sb = [pers.tile([128, D], BF16, tag=f"v{i}", name=f"v{i}")
                for i in range(NKI)]
        yT_sb = [pers.tile([128, S], BF16, tag=f"yT{h}", name=f"yT{h}")
                 for h in range(NHL)]

        psum = top.enter_context(tc.tile_pool(name="psum", bufs=1, space="PSUM"))
        hot = top.enter_context(tc.tile_pool(name="hot", bufs=1))
        wts = top.enter_context(tc.tile_pool(name="wts", bufs=1))

        mw_sb = hot.tile([128, 128], BF16, tag="mw", name="mw_sb")
        onesc = hot.tile([128, 1], BF16, tag="onesc", name="onesc")
        onesr = hot.tile([1, 128], F32R, tag="onesr", name="onesr")
        sin_sb = wts.tile([128, S], BF16, tag="sin", name="sin_sb")
        ident_sb = wts.tile([128, 128], BF16, tag="ident", name="ident_sb")
        swap_sb = wts.tile([128, 128], BF16, tag="swap", name="swap_sb")
        wq = wts.tile([128, EC * NHL * D], BF16, tag="wq", name="wq")
        wk = wts.tile([128, EC * D], BF16, tag="wk", name="wk")
        wv = wts.tile([128, EC * D], BF16, tag="wv", name="wv")
        wo = wts.tile([128, NHL * E], BF16, tag="wo", name="wo")

        xpool = top.enter_context(tc.tile_pool(name="xt", bufs=2))

        # Startup-critical DMA ordering: the first x e-groups and wq feed
        # the first proj chains, so they lead both HWDGE queues in
        # consumption order; later-needed tensors queue behind. Only cheap
        # constants ride the (slow ucode) Pool queue.
        GW = 4 * PW  # columns per x e-group piece
        xt0 = xpool.tile([128, EC * PW], BF16, tag="xt", name="xt0")
        for g in range(4):
            eng = nc.sync if g % 2 == 0 else nc.scalar
            eng.dma_start(xt0[:, GW * g:GW * (g + 1)],
                          xs[0:128, GW * g:GW * (g + 1)])
            eng.dma_start(wq[:, 2048 * g:2048 * (g + 1)],
                          Wq[:, 2048 * g:2048 * (g + 1)])
        nc.sync.dma_start(sin_sb[:], sinT[:])
        nc.scalar.dma_start(wk[:], Wk[:])
        nc.sync.dma_start(wv[:], Wv[:])
        nc.gpsimd.dma_start(swap_sb[:], swapsgn[:])
        nc.gpsimd.dma_start(ident_sb[:], ident[:])
        nc.gpsimd.dma_start(mw_sb[:], mdiag[:])
        nc.gpsimd.dma_start(onesc[:], ones_col[:])
        nc.gpsimd.dma_start(onesr[:], ones_row[:])
        nc.scalar.dma_start(wo[:, 0:NHL * E // 2], Wo[:, 0:NHL * E // 2])
        nc.sync.dma_start(wo[:, NHL * E // 2:], Wo[:, NHL * E // 2:])

        for sp in range(NSP):
            if sp > 0 and reps > 1 and STAGGER:
                tc.stage_boundary()
            lo, hi = PW * sp, PW * (sp + 1)
            W = slice(lo, hi)

            # ---- x slab load (4 consolidated strided DMAs) ----
            if sp == 0:
                xt = xt0
            else:
                xt = xpool.tile([128, EC * PW], BF16, tag="xt", name=f"xt{sp}")
                for g in range(4):
                    nc.sync.dma_start(xt[:, GW * g:GW * (g + 1)],
                                      xs[128 * sp:128 * (sp + 1),
                                         GW * g:GW * (g + 1)])

            def xsl(e):
                return xt[:, PW * e:PW * (e + 1)]

            # ---- QKV projections + RoPE ----
            # Phase A: all six proj chains back-to-back on the PE; each is
            # evacuated by a DVE sin-multiply (the RoPE first term).
            # Phase B: the five rot matmuls + DVE adds. By then every
            # sin-multiply has long finished, so the PE never waits on DVE.
            for h in range(NHL):
                ps = psum.tile([128, PW], F32, tag="proj", bufs=2,
                               name=f"psq{sp}_{h}")
                for e in range(EC):
                    nc.tensor.matmul(
                        ps[:],
                        wq[:, NHL * D * e + D * h:NHL * D * e + D * (h + 1)],
                        xsl(e), start=(e == 0), stop=(e == EC - 1))
                nc.vector.tensor_mul(qT_sb[h][:, W], ps[:], sin_sb[:, W])
            ps = psum.tile([128, PW], F32, tag="proj", bufs=2, name=f"psk{sp}")
            for e in range(EC):
                nc.tensor.matmul(ps[:], wk[:, D * e:D * (e + 1)], xsl(e),
                                 start=(e == 0), stop=(e == EC - 1))
            nc.vector.tensor_mul(kT_sb[:, W], ps[:], sin_sb[:, W])
            ps = psum.tile([128, PW], F32, tag="proj", bufs=2, name=f"psv{sp}")
            for e in range(EC):
                nc.tensor.matmul(ps[:], wv[:, D * e:D * (e + 1)], xsl(e),
                                 start=(e == 0), stop=(e == EC - 1))
            vstage = hot.tile([128, PW], BF16, tag="vstage", bufs=4,
                              name=f"vst{sp}")
            nc.scalar.activation(vstage[:], ps[:], COPY)
            for j in range(4):
                vt = psum.tile([128, 128], BF16, tag="proj", bufs=2,
                               name=f"vtr{sp}_{j}")
                nc.tensor.transpose(vt[:], vstage[:, 128 * j:128 * (j + 1)],
                                    ident_sb[:])
                nc.scalar.activation(v_sb[4 * sp + j][:], vt[:], COPY)

            if KCUT == "proj":
                continue
            # ---- causal attention for q-chunk qj == sp ----
            qj = sp
            nki_hi = (qj + 1) * QC // 128
            # Heads in interleaved PAIRS: per k-tile step the two heads'
            # score matmuls / exps / PV matmuls alternate so one head's ACT
            # exp hides under the other's PE work.
            for hp in (0, 2):
                ypss, rss = [], []
                for h in (hp, hp + 1):
                    ypss.append(psum.tile([128, QC], F32, tag="yps", bufs=2,
                                          name=f"yps{h}_{qj}"))
                    rss.append(hot.tile([128, QC], BF16, tag="rs", bufs=4,
                                        name=f"rs{h}_{qj}"))
                for ki in range(nki_hi):
                    off = 128 * ki - QC * qj
                    qlo = max(0, off)
                    for hj, h in enumerate((hp, hp + 1)):
                        st = psum.tile([128, QC], F32, tag="st", bufs=2,
                                       name=f"st{h}_{qj}_{ki}")
                        nc.tensor.matmul(
                            st[:, qlo:QC], kT_sb[:, 128 * ki:128 * (ki + 1)],
                            qT_sb[h][:, QC * qj + qlo:QC * (qj + 1)],
                            start=True, stop=True)
                        if ki == 0:
                            # exp writes the denominator accumulator
                            # directly (qlo == 0 at ki == 0): no DVE copy
                            pt = rss[hj]
                        else:
                            pt = hot.tile([128, QC], BF16, tag="pt", bufs=8,
                                          name=f"pt{h}_{qj}_{ki}")
                        nc.scalar.activation(pt[:, qlo:QC], st[:, qlo:QC],
                                             EXP, scale=scale)
                        if off >= 0:
                            nc.vector.tensor_mul(pt[:, qlo:qlo + 128],
                                                 pt[:, qlo:qlo + 128],
                                                 mw_sb[:])
                        if ki != 0:
                            nc.vector.tensor_add(rss[hj][:, qlo:QC],
                                                 rss[hj][:, qlo:QC],
                                                 pt[:, qlo:QC])
                        nc.tensor.matmul(ypss[hj][:, qlo:QC], v_sb[ki][:],
                                         pt[:, qlo:QC], start=(ki == 0),
                                         stop=(ki == nki_hi - 1))
                for hj, h in enumerate((hp, hp + 1)):
                    yps, rs = ypss[hj], rss[hj]
                    rsum = psum.tile([1, QC], F32, tag="st", bufs=2,
                                     name=f"rsum{h}_{qj}")
                    nc.tensor.matmul(rsum[:], onesc[:], rs[:],
                                     start=True, stop=True)
                    rinv = hot.tile([1, QC], F32, tag="rinv", bufs=4,
                                    name=f"rinv{h}_{qj}")
                    with nc.allow_low_precision(reason="softmax denominator"):
                        nc.vector.reciprocal_approx_fast(rinv[:], rsum[:])
                    rb = hot.tile([128, QC], F32, tag="rb", bufs=4,
                                  name=f"rb{h}_{qj}")
                    nc.gpsimd.partition_broadcast(rb[:], rinv[:])
                    nc.vector.tensor_mul(yT_sb[h][:, QC * qj:QC * (qj + 1)],
                                         yps[:], rb[:])

            if KCUT == "projattn":
                continue
            # ---- out-proj partial for this chunk ----
            # PSUM is evacuated by ACT copies (ACT has headroom; DVE does
            # not), staged bf16 in SBUF, then one row-store DMA per si.
            for si in range(4 * sp, 4 * sp + 4):
                osb = hot.tile([128, E], BF16, tag="osb", bufs=3,
                               name=f"osb{si}")
                for nj in range(E // 512):
                    ops = psum.tile([128, 512], F32, tag="ops", bufs=2,
                                    name=f"ops{si}_{nj}")
                    for h in range(NHL):
                        nc.tensor.matmul(
                            ops[:], yT_sb[h][:, 128 * si:128 * (si + 1)],
                            wo[:, E * h + 512 * nj:E * h + 512 * (nj + 1)],
                            start=(h == 0), stop=(h == NHL - 1))
                    if nj % 2 == 0:
                        nc.scalar.activation(osb[:, 512 * nj:512 * (nj + 1)],
                                             ops[:], COPY)
                    else:
                        nc.vector.tensor_copy(osb[:, 512 * nj:512 * (nj + 1)],
                                              ops[:])
                nc.sync.dma_start(out[128 * si:128 * (si + 1), :], osb[:])

    nc.compile()
    return nc


def make_consts(S):
    """Host-precomputed constants (rope sin table, masks, permutation)."""
    rope_dim = D // 2
    j = np.arange(rope_dim, dtype=np.float64)
    thetas = 1.0 / ROPE_BASE ** (2.0 * j / rope_dim)
    positions = np.arange(S, dtype=np.float64)
    angles = positions[:, None] * thetas[None, :]
    sin = np.sin(np.concatenate([angles, angles], axis=1)).astype(np.float32)
    sinT = np.ascontiguousarray(sin.T)                       # [D, S]

    k_idx = np.arange(128)[:, None]
    c_idx = np.arange(128)[None, :]
    mdiag = (k_idx <= c_idx).astype(np.float32)

    # swapsgn[p, i] as matmul lhsT: rot[i] = sum_p swapsgn[p, i] * qs[p]
    #   i < 64:  rot[i] = -qs[i + 64]
    #   i >= 64: rot[i] = +qs[i - 64]
    swapsgn = np.zeros((128, 128), np.float32)
    i_lo = np.arange(64)
    swapsgn[i_lo + 64, i_lo] = -1.0
    swapsgn[i_lo, i_lo + 64] = 1.0

    return {
        "sinT": sinT.astype(NPBF16),
        "mdiag": mdiag.astype(NPBF16),
        "ones_col": np.ones((128, 1), np.float32).astype(NPBF16),
        "ones_row": np.ones((1, 128), np.float32),
        "ident": np.eye(128, dtype=np.float32).astype(NPBF16),
        "swapsgn": swapsgn.astype(NPBF16),
    }


def make_in_maps(x, Wq, Wk, Wv, Wo, S, E, QC=512, bo=None):
    """Shard full inputs into the 8 per-core input maps (bf16 on device)."""
    consts = make_consts(S)
    in_maps = []
    for c in range(N_CORES):
        b, g = c // TP, c % TP
        m = dict(consts)
        EC = E // 128
        NSP = S // QC
        # xs[128*sp + p, e*QC + mm] = x[b][QC*sp + mm, 128*e + p]
        xb = x[b].astype(NPBF16)                       # [S, E]
        xs = (xb.reshape(NSP, QC, EC, 128)
                .transpose(0, 3, 2, 1)                 # [NSP, 128, EC, QC]
                .reshape(NSP * 128, EC * QC))
        m["xs"] = np.ascontiguousarray(xs)
        # RoPE rotation folded into the weights: the reference computes
        # q' = q*sin + rot(q)*sin (same sin factor on both terms), so
        # q' = (x @ (W + W@P)) * sin with P the signed half-swap. Applied
        # per 128-wide head block in fp32 before the bf16 cast.
        def fold_rot(Wb):
            Wb = Wb.astype(np.float32)
            out = Wb.copy()
            nb = Wb.shape[1] // D
            for b_ in range(nb):
                blk = Wb[:, D * b_:D * (b_ + 1)]
                rot = np.concatenate([-blk[:, 64:], blk[:, :64]], axis=1)
                out[:, D * b_:D * (b_ + 1)] = blk + rot
            return out

        # wq[p, e*(4D) + n] = Wq'[128*e + p, 4D*g + n]
        wqs = (fold_rot(Wq[:, NHL * D * g:NHL * D * (g + 1)]).astype(NPBF16)
               .reshape(EC, 128, NHL * D).transpose(1, 0, 2)
               .reshape(128, EC * NHL * D))
        m["Wq"] = np.ascontiguousarray(wqs)
        wks = (fold_rot(Wk[:, D * g:D * (g + 1)]).astype(NPBF16)
               .reshape(EC, 128, D).transpose(1, 0, 2).reshape(128, EC * D))
        m["Wk"] = np.ascontiguousarray(wks)
        wvs = (Wv[:, D * g:D * (g + 1)].astype(NPBF16)
               .reshape(EC, 128, D).transpose(1, 0, 2).reshape(128, EC * D))
        m["Wv"] = np.ascontiguousarray(wvs)
        # wo[p, h*E + n] = Wo[4D*g + 128*h + p, n]
        wos = (Wo[NHL * D * g:NHL * D * (g + 1), :].astype(NPBF16)
               .reshape(NHL, 128, E).transpose(1, 0, 2).reshape(128, NHL * E))
        m["Wo"] = np.ascontiguousarray(wos)
        in_maps.append(m)
    return in_maps


_CACHE = {}


def _compiled_full():
    if "nc" not in _CACHE:
        _CACHE["nc"] = build_program(S_FULL, E_FULL)
    return _CACHE["nc"]


def kernel(x, Wq, Wk, Wv, Wo, bo):
    nc = _compiled_full()
    in_maps = make_in_maps(x, Wq, Wk, Wv, Wo, S_FULL, E_FULL, bo=bo)
    res = run_bass_kernel_spmd(nc, in_maps, list(range(N_CORES)))
    # unshard the row-parallel out-proj: sum the 4 head-group partials + bias
    out = np.zeros((BATCH, S_FULL, E_FULL), np.float32)
    for c in range(N_CORES):
        out[c // TP] += res.results[c]["out"].astype(np.float32)
    out += bo.astype(np.float32)[None, None, :]
    return out
